# revision 1
# baseline (speedup 1.0000x reference)
"""Complex multi-head attention on 8 Trainium2 cores (Bass/Tile).

Sharding: pure data-parallel over batch (B=8 -> 1 batch per core),
weights replicated. No collectives.

Per-core dataflow (batch b), all matmuls float32r (full rate at N=512):
  - Host supplies feature-major activations XT = [xr.T; xi.T] [1024, S]
    and repacked/sign-folded weights so every complex linear is one
    stacked-K real matmul chain.
  - V-projection (all heads) -> V1 token-major [t, (h, vr|vi)].
  - Per head h: Q/K projections -> feature-major stacks [(c,dh)=128, S];
    scores computed TRANSPOSED (S.T = K-stationary) so softmax'd scores
    feed the AV matmul directly (no transposes anywhere);
    softmax without max-subtraction (|s| <= ~16, exp safe in fp32);
    row sums via ones-matmuls packed into one PSUM bank (tile_position);
    normalization fused into the P1/P2 PSUM evacuation via
    broadcast-DMA'd reciprocals.
  - Output projection accumulates heads as K-chunks -> [t, (o, c)] which
    is exactly the [S, D, 2] DRAM layout.
"""

import sys
import types
import numpy as np

B, S, D, H = 8, 1024, 512, 8
DH = D // H
KC = 8  # k-chunks of 128 over (c,d) = 1024
TC = 8  # token chunks of 128
NCORES = 8

LAST_EXEC_NS = None


# ---------------------------------------------------------------- shims
def _install_axon_profile_shim():
    if "antenv.axon_hooks" in sys.modules:
        return
    try:
        import antenv  # noqa: F401

        mod = types.ModuleType("antenv.axon_hooks")
        state = {"hook": None}
        mod.set_axon_ntff_profile_hook = lambda h: state.__setitem__("hook", h)
        mod.get_axon_ntff_profile_hook = lambda: state["hook"]
        sys.modules["antenv.axon_hooks"] = mod
        from trn_agent_boot.trn_boot import _ntff_profile_via_ctypes

        hook = _ntff_profile_via_ctypes("/opt/axon/libaxon_pjrt.so")
        if hook is not None:
            mod.set_axon_ntff_profile_hook(hook)
    except Exception:
        pass


def _install_tile_drain_patch():
    """This walrus build allows ONE sync wait per instruction; split the
    TileContext exit drain's waits across preceding sync NOPs."""
    import concourse.mybir as mybir
    import concourse.tile as tile
    from concourse.vector_clock import ScopedClock

    if getattr(tile.TileContext, "_drain_patched", False):
        return

    def _patched(self, tick_clock, wait_clock):
        probe = mybir.InstNoOp(name="I-drain-probe")
        probe.engine = mybir.EngineType.SP
        wait_clock.add_sem_waits(probe, ScopedClock({None: tick_clock.global_clock}))
        waits = list(probe.sync_info.on_wait or []) if probe.sync_info else []
        for w in waits:
            nop = self.nc.sync.nop()
            nop.ins.sync_info = mybir.SyncInfo(on_wait=[w], on_update=[])
        self.nc.sync.drain()
        self.nc.all_engine_barrier()
        assert self.sems is not None
        popped = self.nc._tile_sem_poison_stack.pop()
        assert popped is self._sem_poison
        self.nc.clear_and_free_semaphores(list(self.sems.allocated().values()))
        self.nc.all_engine_barrier()

    tile.TileContext._drain_and_barrier = _patched
    tile.TileContext._drain_patched = True


def _split_waits(nc, max_waits=1):
    """Hoist extra sync waits onto preceding same-engine NOPs (walrus here
    rejects >1 sync wait per instruction)."""
    import concourse.mybir as mybir

    def process(blk):
        lst = blk.instructions
        i = 0
        while i < len(lst):
            inst = lst[i]
            if hasattr(inst, "blocks"):
                for b in inst.blocks or []:
                    process(b)
            si = inst.sync_info
            if si is not None and si.on_wait and len(si.on_wait) > max_waits:
                waits = list(si.on_wait)
                keep, extra = waits[-max_waits:], waits[:-max_waits]
                inst.sync_info = mybir.SyncInfo(
                    on_wait=keep, on_update=list(si.on_update or [])
                )
                for j, w in enumerate(extra):
                    nop = mybir.InstNoOp(name=f"{inst.name}-ws{j}")
                    nop.engine = inst.engine
                    nop.sync_info = mybir.SyncInfo(on_wait=[w], on_update=[])
                    lst.insert(i, nop)
                    i += 1
            i += 1

    for f in nc.m.functions:
        for blk in f.blocks:
            process(blk)


# ------------------------------------------------------------ host prep
def _build_wqk(wr, wi, scale):
    """[1024 k=(c,d), 1024 m=(h, c', dh)] for Q/K projections."""
    W = np.empty((2 * D, 2 * D), np.float32)
    for h in range(H):
        o = slice(h * DH, (h + 1) * DH)
        c0 = h * 2 * DH
        W[0:D, c0 : c0 + DH] = wr[o].T * scale
        W[D:, c0 : c0 + DH] = -wi[o].T * scale
        W[0:D, c0 + DH : c0 + 2 * DH] = wi[o].T * scale
        W[D:, c0 + DH : c0 + 2 * DH] = wr[o].T * scale
    return W


def _head_tiles(W):
    """[1024,1024] -> [H, 128, 1024]: per-head column block, k-chunk cols."""
    out = np.empty((H, 128, 1024), np.float32)
    for h in range(H):
        blk = W[:, h * 128 : (h + 1) * 128]  # [1024, 128]
        for kk in range(KC):
            out[h, :, kk * 128 : (kk + 1) * 128] = blk[kk * 128 : (kk + 1) * 128]
    return out


def _kchunk_tiles(W):
    """[1024,1024] -> [KC, 128, 1024]: row chunks."""
    return np.ascontiguousarray(W.reshape(KC, 128, 1024))


def _build_wo(wo_r, wo_i):
    """rows (h, c', dh), cols (o, c) interleaved to match [S, D, 2]."""
    W = np.empty((2 * D, 2 * D), np.float32)
    for h in range(H):
        d = slice(h * DH, (h + 1) * DH)
        r0 = h * 2 * DH
        W[r0 : r0 + DH, 0::2] = wo_r[:, d].T
        W[r0 : r0 + DH, 1::2] = wo_i[:, d].T
        W[r0 + DH : r0 + 2 * DH, 0::2] = -wo_i[:, d].T
        W[r0 + DH : r0 + 2 * DH, 1::2] = wo_r[:, d].T
    return W


def _xt(x):  # [S, D, 2] -> [2D, S] feature-major
    out = np.empty((2 * D, S), np.float32)
    out[0:D] = x[:, :, 0].T
    out[D:] = x[:, :, 1].T
    return out


# ------------------------------------------------------------ bass build
def _build_nc():
    import concourse.bass as bass
    import concourse.bass as bass_mod
    import concourse.mybir as mybir
    import concourse.tile as tile
    from contextlib import ExitStack

    MDT = mybir.dt.float32r
    F32 = mybir.dt.float32

    nc = bass.Bass()
    d_xtq = nc.dram_tensor("xtq", [KC, 128, S], MDT, kind="ExternalInput")
    d_xtk = nc.dram_tensor("xtk", [KC, 128, S], MDT, kind="ExternalInput")
    d_xtv = nc.dram_tensor("xtv", [KC, 128, S], MDT, kind="ExternalInput")
    d_wq = nc.dram_tensor("wq", [H, 128, 1024], MDT, kind="ExternalInput")
    d_wk = nc.dram_tensor("wk", [H, 128, 1024], MDT, kind="ExternalInput")
    d_wv = nc.dram_tensor("wv", [KC, 128, 1024], MDT, kind="ExternalInput")
    d_wo = nc.dram_tensor("wo", [H, 128, 1024], MDT, kind="ExternalInput")
    d_cst = nc.dram_tensor("cst", [128, 320], MDT, kind="ExternalInput")
    d_out = nc.dram_tensor("out", [S, 1024], F32, kind="ExternalOutput")

    with tile.TileContext(nc) as tc, ExitStack() as ctx:
        ctx.enter_context(
            nc.allow_low_precision(reason="float32r tiles are bit-identical fp32")
        )
        pA = ctx.enter_context(tc.tile_pool(name="bigA", bufs=8))  # xtv -> xtq -> wo
        pB = ctx.enter_context(tc.tile_pool(name="bigB", bufs=8))  # wv -> xtk -> oev
        pV1 = ctx.enter_context(tc.tile_pool(name="v1", bufs=8))
        pOsb = ctx.enter_context(tc.tile_pool(name="osb", bufs=8))
        pWqk = ctx.enter_context(tc.tile_pool(name="wqk", bufs=3))
        pStk = ctx.enter_context(tc.tile_pool(name="stk", bufs=5))
        pE = ctx.enter_context(tc.tile_pool(name="e", bufs=4))
        pV2 = ctx.enter_context(tc.tile_pool(name="v2", bufs=16))
        pSm = ctx.enter_context(tc.tile_pool(name="sm", bufs=6))
        pRec = ctx.enter_context(tc.tile_pool(name="rec", bufs=1))  # rec/bc/tmp
        pC = ctx.enter_context(tc.tile_pool(name="const", bufs=1))

        ps_st = ctx.enter_context(tc.tile_pool(name="ps_st", bufs=4, space="PSUM"))
        ps_p12 = ctx.enter_context(tc.tile_pool(name="ps_p12", bufs=2, space="PSUM"))
        ps_sums = ctx.enter_context(tc.tile_pool(name="ps_sums", bufs=2, space="PSUM"))
        ps_proj = ps_st

        cst = pC.tile([128, 320], MDT, tag="cst")
        nc.sync.dma_start(out=cst, in_=d_cst[:, :])
        ones128 = cst[:, 0:128]

        # ---- phase V: V projection (all heads) ----
        xtv = []
        for kk in range(KC):
            t = pA.tile([128, S], MDT, tag="bigA")
            nc.sync.dma_start(out=t, in_=d_xtv[kk])
            xtv.append(t)
        wv = []
        for kk in range(KC):
            t = pB.tile([128, 1024], MDT, tag="bigB")
            nc.sync.dma_start(out=t, in_=d_wv[kk])
            wv.append(t)
        v1 = []
        for t_ in range(TC):
            vt = pV1.tile([128, 1024], MDT, tag="v1")
            for nh in range(2):
                ps = ps_st.tile([128, 512], F32, tag="ps_st")
                for kk in range(KC):
                    nc.tensor.matmul(
                        ps,
                        lhsT=xtv[kk][:, t_ * 128 : (t_ + 1) * 128],
                        rhs=wv[kk][:, nh * 512 : (nh + 1) * 512],
                        start=(kk == 0),
                        stop=(kk == KC - 1),
                    )
                nc.vector.tensor_copy(vt[:, nh * 512 : (nh + 1) * 512], ps)
            v1.append(vt)

        # ---- load XT_q / XT_k (reuse pA / pB slots) ----
        xtq, xtk = [], []
        for kk in range(KC):
            t = pA.tile([128, S], MDT, tag="bigA")
            nc.sync.dma_start(out=t, in_=d_xtq[kk])
            xtq.append(t)
        for kk in range(KC):
            t = pB.tile([128, S], MDT, tag="bigB")
            nc.sync.dma_start(out=t, in_=d_xtk[kk])
            xtk.append(t)

        # ---- attention per head ----
        osb = []
        for h in range(H):
            wqh = pWqk.tile([128, 1024], MDT, tag="wqk")
            nc.sync.dma_start(out=wqh, in_=d_wq[h])
            wkh = pWqk.tile([128, 1024], MDT, tag="wqk")
            nc.sync.dma_start(out=wkh, in_=d_wk[h])

            # Q projection -> qstack [(c,dh)=128, S]
            qstack = pStk.tile([128, S], MDT, tag="stk")
            for nh in range(2):
                ps = ps_proj.tile([128, 512], F32, tag="ps_st")
                for kk in range(KC):
                    nc.tensor.matmul(
                        ps,
                        lhsT=wqh[:, kk * 128 : (kk + 1) * 128],
                        rhs=xtq[kk][:, nh * 512 : (nh + 1) * 512],
                        start=(kk == 0),
                        stop=(kk == KC - 1),
                    )
                nc.vector.tensor_copy(qstack[:, nh * 512 : (nh + 1) * 512], ps)
            # qswap = [qi.T; qr.T] via partition-crossing SBUF->SBUF DMA
            qswap = pStk.tile([128, S], MDT, tag="stk")
            nc.sync.dma_start(out=qswap[0:64, :], in_=qstack[64:128, :])
            nc.sync.dma_start(out=qswap[64:128, :], in_=qstack[0:64, :])

            # K projection -> kstack [kr.T; ki.T], kneg [kr.T; -ki.T]
            kstack = pStk.tile([128, S], MDT, tag="stk")
            kneg = pStk.tile([128, S], MDT, tag="stk")
            for nh in range(2):
                sl = slice(nh * 512, (nh + 1) * 512)
                ps = ps_proj.tile([128, 512], F32, tag="ps_st")
                for kk in range(KC):
                    nc.tensor.matmul(
                        ps,
                        lhsT=wkh[:, kk * 128 : (kk + 1) * 128],
                        rhs=xtk[kk][:, nh * 512 : (nh + 1) * 512],
                        start=(kk == 0),
                        stop=(kk == KC - 1),
                    )
                nc.vector.tensor_copy(kstack[:, sl], ps)
                nc.vector.tensor_copy(kneg[0:64, sl], ps[0:64, :])
                nc.vector.tensor_scalar_mul(kneg[64:128, sl], ps[64:128, :], -1.0)

            # V2_h tiles: [-vi | vr] per tk-chunk
            v2h = []
            for tk in range(TC):
                vt = pV2.tile([128, 128], MDT, tag="v2")
                base = h * 128
                nc.vector.tensor_scalar_mul(
                    vt[:, 0:64], v1[tk][:, base + 64 : base + 128], -1.0
                )
                nc.vector.tensor_copy(vt[:, 64:128], v1[tk][:, base : base + 64])
                v2h.append(vt)

            ot = pOsb.tile([128, S], MDT, tag="osb")
            for nh in range(2):
                nsl = slice(nh * 512, (nh + 1) * 512)
                sums_r = ps_sums.tile([128, 512], F32, tag="ps_sums")
                sums_i = ps_sums.tile([128, 512], F32, tag="ps_sums")
                p1 = ps_p12.tile([128, 512], F32, tag="ps_p12")
                p2 = ps_p12.tile([128, 512], F32, tag="ps_p12")
                for tk in range(TC):
                    ksl = slice(tk * 128, (tk + 1) * 128)
                    for comp in range(2):  # 0: real scores, 1: imag scores
                        lhsT_k = kneg if comp == 0 else kstack
                        rhs_q = qstack if comp == 0 else qswap
                        pdst = p1 if comp == 0 else p2
                        sdst = sums_r if comp == 0 else sums_i
                        vt = v1[tk][:, h * 128 : (h + 1) * 128] if comp == 0 else v2h[tk]
                        st = ps_st.tile([128, 512], F32, tag="ps_st")
                        nc.tensor.matmul(
                            st,
                            lhsT=lhsT_k[:, ksl],
                            rhs=rhs_q[:, nsl],
                            start=True,
                            stop=True,
                        )
                        e = pE.tile([128, 512], MDT, tag="e")
                        nc.scalar.activation(
                            e, st, func=mybir.ActivationFunctionType.Exp
                        )
                        nc.tensor.matmul(
                            sdst,
                            lhsT=ones128,
                            rhs=e,
                            start=(tk == 0),
                            stop=(tk == TC - 1),
                        )
                        nc.tensor.matmul(
                            pdst,
                            lhsT=vt,
                            rhs=e,
                            start=(tk == 0),
                            stop=(tk == TC - 1),
                        )
                rec_r = pSm.tile([128, 512], MDT, tag="sm")
                nc.vector.reciprocal(rec_r, sums_r)
                rec_i = pSm.tile([128, 512], MDT, tag="sm")
                nc.vector.reciprocal(rec_i, sums_i)
                t1 = pSm.tile([128, 512], F32, tag="sm")
                t2 = pSm.tile([128, 512], F32, tag="sm")
                nc.vector.tensor_mul(t1, p1, rec_r)
                nc.vector.tensor_mul(t2, p2, rec_i)
                nc.vector.tensor_add(ot[:, nsl], t1, t2)
            osb.append(ot)

        # ---- output projection ----
        wo = []
        for h in range(H):
            t = pA.tile([128, 1024], MDT, tag="bigA")
            nc.sync.dma_start(out=t, in_=d_wo[h])
            wo.append(t)
        for t_ in range(TC):
            tsl = slice(t_ * 128, (t_ + 1) * 128)
            for nh in range(2):
                nsl = slice(nh * 512, (nh + 1) * 512)
                ps = ps_st.tile([128, 512], F32, tag="ps_st")
                for h in range(H):
                    nc.tensor.matmul(
                        ps,
                        lhsT=osb[h][:, tsl],
                        rhs=wo[h][:, nsl],
                        start=(h == 0),
                        stop=(h == H - 1),
                    )
                oev = pB.tile([128, 512], F32, tag="bigB")
                nc.scalar.copy(oev, ps)
                nc.sync.dma_start(out=d_out[tsl, nsl], in_=oev)

    _split_waits(nc)
    return nc


_NC_CACHE = {}


def kernel(
    queries,
    keys,
    values,
    wq_r,
    wq_i,
    wk_r,
    wk_i,
    wv_r,
    wv_i,
    wo_r,
    wo_i,
    _trace=False,
):
    global LAST_EXEC_NS
    _install_axon_profile_shim()
    _install_tile_drain_patch()
    from concourse.bass_utils import run_bass_kernel_spmd

    scale = 1.0 / np.sqrt(DH)
    WQ = _head_tiles(_build_wqk(np.asarray(wq_r), np.asarray(wq_i), scale))
    WK = _head_tiles(_build_wqk(np.asarray(wk_r), np.asarray(wk_i), 1.0))
    WV = _kchunk_tiles(_build_wqk(np.asarray(wv_r), np.asarray(wv_i), 1.0))
    WO = _kchunk_tiles(_build_wo(np.asarray(wo_r), np.asarray(wo_i)))
    CST = np.zeros((128, 320), np.float32)
    CST[:, 0:128] = 1.0

    queries = np.asarray(queries)
    keys = np.asarray(keys)
    values = np.asarray(values)

    in_maps = []
    for b in range(NCORES):
        in_maps.append(
            {
                "xtq": _xt(queries[b]).reshape(KC, 128, S),
                "xtk": _xt(keys[b]).reshape(KC, 128, S),
                "xtv": _xt(values[b]).reshape(KC, 128, S),
                "wq": WQ,
                "wk": WK,
                "wv": WV,
                "wo": WO,
                "cst": CST,
            }
        )

    if "nc" not in _NC_CACHE:
        _NC_CACHE["nc"] = _build_nc()
    nc = _NC_CACHE["nc"]

    res = run_bass_kernel_spmd(nc, in_maps, list(range(NCORES)), trace=_trace)
    LAST_EXEC_NS = res.exec_time_ns

    out = np.empty((B, S, D, 2), np.float32)
    for b in range(NCORES):
        out[b] = res.results[b]["out"].reshape(S, D, 2)
    return out



# revision 12
# speedup vs baseline: 1.1652x; 1.1652x over previous
"""Complex multi-head attention on 8 Trainium2 cores (Bass/Tile).

Sharding: pure data-parallel over batch (B=8 -> 1 batch per core),
weights replicated. No collectives.

Per-core dataflow (batch b), all matmuls float32r (full rate at N=512):
  - Host supplies feature-major activations XT = [xr.T; xi.T] [1024, S]
    and repacked/sign-folded weights so every complex linear is one
    stacked-K real matmul chain.
  - V-projection (all heads) -> V1 token-major [t, (h, vr|vi)].
  - Per head h: Q/K projections -> feature-major stacks [(c,dh)=128, S];
    scores computed TRANSPOSED (S.T = K-stationary) so softmax'd scores
    feed the AV matmul directly (no transposes anywhere);
    softmax without max-subtraction (|s| <= ~16, exp safe in fp32);
    row sums via ones-matmuls packed into one PSUM bank (tile_position);
    normalization fused into the P1/P2 PSUM evacuation via
    broadcast-DMA'd reciprocals.
  - Output projection accumulates heads as K-chunks -> [t, (o, c)] which
    is exactly the [S, D, 2] DRAM layout.
"""

import sys
import types
import numpy as np

B, S, D, H = 8, 1024, 512, 8
DH = D // H
KC = 8  # k-chunks of 128 over (c,d) = 1024
TC = 8  # token chunks of 128
NCORES = 8

LAST_EXEC_NS = None


# ---------------------------------------------------------------- shims
def _install_axon_profile_shim():
    if "antenv.axon_hooks" in sys.modules:
        return
    try:
        import antenv  # noqa: F401

        mod = types.ModuleType("antenv.axon_hooks")
        state = {"hook": None}
        mod.set_axon_ntff_profile_hook = lambda h: state.__setitem__("hook", h)
        mod.get_axon_ntff_profile_hook = lambda: state["hook"]
        sys.modules["antenv.axon_hooks"] = mod
        from trn_agent_boot.trn_boot import _ntff_profile_via_ctypes

        hook = _ntff_profile_via_ctypes("/opt/axon/libaxon_pjrt.so")
        if hook is not None:
            mod.set_axon_ntff_profile_hook(hook)
    except Exception:
        pass


def _install_tile_drain_patch():
    """This walrus build allows ONE sync wait per instruction; split the
    TileContext exit drain's waits across preceding sync NOPs."""
    import concourse.mybir as mybir
    import concourse.tile as tile
    from concourse.vector_clock import ScopedClock

    if getattr(tile.TileContext, "_drain_patched", False):
        return

    def _patched(self, tick_clock, wait_clock):
        probe = mybir.InstNoOp(name="I-drain-probe")
        probe.engine = mybir.EngineType.SP
        wait_clock.add_sem_waits(probe, ScopedClock({None: tick_clock.global_clock}))
        waits = list(probe.sync_info.on_wait or []) if probe.sync_info else []
        for w in waits:
            nop = self.nc.sync.nop()
            nop.ins.sync_info = mybir.SyncInfo(on_wait=[w], on_update=[])
        self.nc.sync.drain()
        self.nc.all_engine_barrier()
        assert self.sems is not None
        popped = self.nc._tile_sem_poison_stack.pop()
        assert popped is self._sem_poison
        self.nc.clear_and_free_semaphores(list(self.sems.allocated().values()))
        self.nc.all_engine_barrier()

    tile.TileContext._drain_and_barrier = _patched
    tile.TileContext._drain_patched = True


def _split_waits(nc, max_waits=1):
    """Hoist extra sync waits onto preceding same-engine NOPs (walrus here
    rejects >1 sync wait per instruction)."""
    import concourse.mybir as mybir

    def process(blk):
        lst = blk.instructions
        i = 0
        while i < len(lst):
            inst = lst[i]
            if hasattr(inst, "blocks"):
                for b in inst.blocks or []:
                    process(b)
            si = inst.sync_info
            if si is not None and si.on_wait and len(si.on_wait) > max_waits:
                waits = list(si.on_wait)
                keep, extra = waits[-max_waits:], waits[:-max_waits]
                inst.sync_info = mybir.SyncInfo(
                    on_wait=keep, on_update=list(si.on_update or [])
                )
                for j, w in enumerate(extra):
                    nop = mybir.InstNoOp(name=f"{inst.name}-ws{j}")
                    nop.engine = inst.engine
                    nop.sync_info = mybir.SyncInfo(on_wait=[w], on_update=[])
                    lst.insert(i, nop)
                    i += 1
            i += 1

    for f in nc.m.functions:
        for blk in f.blocks:
            process(blk)


# ------------------------------------------------------------ host prep
def _build_wqk(wr, wi, scale):
    """[1024 k=(c,d), 1024 m=(h, c', dh)] for Q/K projections."""
    W = np.empty((2 * D, 2 * D), np.float32)
    for h in range(H):
        o = slice(h * DH, (h + 1) * DH)
        c0 = h * 2 * DH
        W[0:D, c0 : c0 + DH] = wr[o].T * scale
        W[D:, c0 : c0 + DH] = -wi[o].T * scale
        W[0:D, c0 + DH : c0 + 2 * DH] = wi[o].T * scale
        W[D:, c0 + DH : c0 + 2 * DH] = wr[o].T * scale
    return W


def _head_tiles(W):
    """[1024,1024] -> [H, 128, 1024]: per-head column block, k-chunk cols."""
    out = np.empty((H, 128, 1024), np.float32)
    for h in range(H):
        blk = W[:, h * 128 : (h + 1) * 128]  # [1024, 128]
        for kk in range(KC):
            out[h, :, kk * 128 : (kk + 1) * 128] = blk[kk * 128 : (kk + 1) * 128]
    return out


def _kchunk_tiles(W):
    """[1024,1024] -> [KC, 128, 1024]: row chunks."""
    return np.ascontiguousarray(W.reshape(KC, 128, 1024))


def _build_wo(wo_r, wo_i):
    """rows (h, c', dh), cols (o, c) interleaved to match [S, D, 2]."""
    W = np.empty((2 * D, 2 * D), np.float32)
    for h in range(H):
        d = slice(h * DH, (h + 1) * DH)
        r0 = h * 2 * DH
        W[r0 : r0 + DH, 0::2] = wo_r[:, d].T
        W[r0 : r0 + DH, 1::2] = wo_i[:, d].T
        W[r0 + DH : r0 + 2 * DH, 0::2] = -wo_i[:, d].T
        W[r0 + DH : r0 + 2 * DH, 1::2] = wo_r[:, d].T
    return W


def _xt(x):  # [S, D, 2] -> [2D, S] feature-major
    out = np.empty((2 * D, S), np.float32)
    out[0:D] = x[:, :, 0].T
    out[D:] = x[:, :, 1].T
    return out


# ------------------------------------------------------------ bass build
def _build_nc():
    import concourse.bass as bass
    import concourse.bass as bass_mod
    import concourse.mybir as mybir
    import concourse.tile as tile
    from contextlib import ExitStack

    MDT = mybir.dt.float32r
    F32 = mybir.dt.float32

    nc = bass.Bass()
    d_xtq = nc.dram_tensor("xtq", [KC, 128, S], MDT, kind="ExternalInput")
    d_xtk = nc.dram_tensor("xtk", [KC, 128, S], MDT, kind="ExternalInput")
    d_xtv = nc.dram_tensor("xtv", [KC, 128, S], MDT, kind="ExternalInput")
    d_wq = nc.dram_tensor("wq", [H, 128, 1024], MDT, kind="ExternalInput")
    d_wk = nc.dram_tensor("wk", [H, 128, 1024], MDT, kind="ExternalInput")
    d_wv = nc.dram_tensor("wv", [KC, 128, 1024], MDT, kind="ExternalInput")
    d_wo = nc.dram_tensor("wo", [H, 128, 1024], MDT, kind="ExternalInput")
    d_cst = nc.dram_tensor("cst", [128, 320], MDT, kind="ExternalInput")
    d_out = nc.dram_tensor("out", [S, 1024], F32, kind="ExternalOutput")

    with tile.TileContext(nc) as tc, ExitStack() as ctx:
        ctx.enter_context(
            nc.allow_low_precision(reason="float32r tiles are bit-identical fp32")
        )
        pA = ctx.enter_context(tc.tile_pool(name="bigA", bufs=8))  # xtv -> xtq -> wo
        pB = ctx.enter_context(tc.tile_pool(name="bigB", bufs=8))  # wv -> xtk -> oev
        pV1 = ctx.enter_context(tc.tile_pool(name="v1", bufs=8))
        pOsb = ctx.enter_context(tc.tile_pool(name="osb", bufs=8))
        pWqk = ctx.enter_context(tc.tile_pool(name="wqk", bufs=3))
        pStk = ctx.enter_context(tc.tile_pool(name="stk", bufs=5))
        pE = ctx.enter_context(tc.tile_pool(name="e", bufs=4))
        pV2 = ctx.enter_context(tc.tile_pool(name="v2", bufs=10))
        pSm = ctx.enter_context(tc.tile_pool(name="sm", bufs=12))
        pC = ctx.enter_context(tc.tile_pool(name="const", bufs=1))

        # PSUM: 8 banks. st pipeline (3, lead-2 software pipeline), attention
        # accumulators p1/p2 (3: one slack bank absorbs the lazy norm),
        # sums + every projection psum (2).
        ps_st = ctx.enter_context(tc.tile_pool(name="ps_st", bufs=3, space="PSUM"))
        ps_p12 = ctx.enter_context(tc.tile_pool(name="ps_p12", bufs=3, space="PSUM"))
        ps_sums = ctx.enter_context(tc.tile_pool(name="ps_sums", bufs=2, space="PSUM"))
        ps_proj = ps_sums

        cst = pC.tile([128, 320], MDT, tag="cst")
        nc.sync.dma_start(out=cst, in_=d_cst[:, :])
        ones128 = cst[:, 0:128]

        # ---- phase V: V projection (all heads) ----
        xtv = []
        for kk in range(KC):
            t = pA.tile([128, S], MDT, tag="bigA")
            nc.sync.dma_start(out=t, in_=d_xtv[kk])
            xtv.append(t)
        wv = []
        for kk in range(KC):
            t = pB.tile([128, 1024], MDT, tag="bigB")
            nc.sync.dma_start(out=t, in_=d_wv[kk])
            wv.append(t)
        v1 = []
        for t_ in range(TC):
            vt = pV1.tile([128, 1024], MDT, tag="v1")
            for nh in range(2):
                ps = ps_proj.tile([128, 512], F32, tag="ps_sums")
                for kk in range(KC):
                    nc.tensor.matmul(
                        ps,
                        lhsT=xtv[kk][:, t_ * 128 : (t_ + 1) * 128],
                        rhs=wv[kk][:, nh * 512 : (nh + 1) * 512],
                        start=(kk == 0),
                        stop=(kk == KC - 1),
                    )
                nc.vector.tensor_copy(vt[:, nh * 512 : (nh + 1) * 512], ps)
            v1.append(vt)

        # ---- load XT_q / XT_k (reuse pA / pB slots) ----
        xtq, xtk = [], []
        for kk in range(KC):
            t = pA.tile([128, S], MDT, tag="bigA")
            nc.sync.dma_start(out=t, in_=d_xtq[kk])
            xtq.append(t)
        for kk in range(KC):
            t = pB.tile([128, S], MDT, tag="bigB")
            nc.sync.dma_start(out=t, in_=d_xtk[kk])
            xtk.append(t)

        # ---- attention per head ----
        osb = []
        for h in range(H):
            wqh = pWqk.tile([128, 1024], MDT, tag="wqk")
            nc.sync.dma_start(out=wqh, in_=d_wq[h])
            wkh = pWqk.tile([128, 1024], MDT, tag="wqk")
            nc.sync.dma_start(out=wkh, in_=d_wk[h])

            # Q projection -> qstack [(c,dh)=128, S]
            qstack = pStk.tile([128, S], MDT, tag="stk")
            for nh in range(2):
                ps = ps_proj.tile([128, 512], F32, tag="ps_sums")
                for kk in range(KC):
                    nc.tensor.matmul(
                        ps,
                        lhsT=wqh[:, kk * 128 : (kk + 1) * 128],
                        rhs=xtq[kk][:, nh * 512 : (nh + 1) * 512],
                        start=(kk == 0),
                        stop=(kk == KC - 1),
                    )
                nc.vector.tensor_copy(qstack[:, nh * 512 : (nh + 1) * 512], ps)
            # qswap = [qi.T; qr.T] via partition-crossing SBUF->SBUF DMA
            qswap = pStk.tile([128, S], MDT, tag="stk")
            nc.sync.dma_start(out=qswap[0:64, :], in_=qstack[64:128, :])
            nc.sync.dma_start(out=qswap[64:128, :], in_=qstack[0:64, :])

            # K projection -> kstack [kr.T; ki.T], kneg [kr.T; -ki.T]
            kstack = pStk.tile([128, S], MDT, tag="stk")
            kneg = pStk.tile([128, S], MDT, tag="stk")
            for nh in range(2):
                sl = slice(nh * 512, (nh + 1) * 512)
                ps = ps_proj.tile([128, 512], F32, tag="ps_sums")
                for kk in range(KC):
                    nc.tensor.matmul(
                        ps,
                        lhsT=wkh[:, kk * 128 : (kk + 1) * 128],
                        rhs=xtk[kk][:, nh * 512 : (nh + 1) * 512],
                        start=(kk == 0),
                        stop=(kk == KC - 1),
                    )
                nc.vector.tensor_copy(kstack[:, sl], ps)
                nc.vector.tensor_copy(kneg[0:64, sl], ps[0:64, :])
                nc.vector.tensor_scalar_mul(kneg[64:128, sl], ps[64:128, :], -1.0)

            # V2_h tiles: [-vi | vr] per tk-chunk
            v2h = []
            for tk in range(TC):
                vt = pV2.tile([128, 128], MDT, tag="v2")
                base = h * 128
                nc.vector.tensor_scalar_mul(
                    vt[:, 0:64], v1[tk][:, base + 64 : base + 128], -1.0
                )
                nc.vector.tensor_copy(vt[:, 64:128], v1[tk][:, base : base + 64])
                v2h.append(vt)

            ot = pOsb.tile([128, S], MDT, tag="osb")
            norm = []  # deferred per-nh normalization state
            for nh in range(2):
                nsl = slice(nh * 512, (nh + 1) * 512)
                sums_r = ps_sums.tile([128, 512], F32, tag="ps_sums")
                sums_i = ps_sums.tile([128, 512], F32, tag="ps_sums")
                p1 = ps_p12.tile([128, 512], F32, tag="ps_p12")
                p2 = ps_p12.tile([128, 512], F32, tag="ps_p12")

                # software-pipelined: score matmuls run LEAD iterations ahead
                # so the in-order tensor queue never blocks on Exp latency.
                iters = [(tk, comp) for tk in range(TC) for comp in range(2)]
                st_tiles = [None] * len(iters)

                def emit_st(k):
                    tk, comp = iters[k]
                    ksl = slice(tk * 128, (tk + 1) * 128)
                    st = ps_st.tile([128, 512], F32, tag="ps_st")
                    nc.tensor.matmul(
                        st,
                        lhsT=(kneg if comp == 0 else kstack)[:, ksl],
                        rhs=(qstack if comp == 0 else qswap)[:, nsl],
                        start=True,
                        stop=True,
                    )
                    st_tiles[k] = st

                LEAD = 2
                for k in range(LEAD):
                    emit_st(k)
                for k in range(len(iters)):
                    tk, comp = iters[k]
                    e = pE.tile([128, 512], MDT, tag="e")
                    nc.scalar.activation(
                        e, st_tiles[k], func=mybir.ActivationFunctionType.Exp
                    )
                    st_tiles[k] = None
                    sdst = sums_r if comp == 0 else sums_i
                    pdst = p1 if comp == 0 else p2
                    vt = v1[tk][:, h * 128 : (h + 1) * 128] if comp == 0 else v2h[tk]
                    nc.tensor.matmul(
                        sdst,
                        lhsT=ones128,
                        rhs=e,
                        start=(tk == 0),
                        stop=(tk == TC - 1),
                    )
                    nc.tensor.matmul(
                        pdst,
                        lhsT=vt,
                        rhs=e,
                        start=(tk == 0),
                        stop=(tk == TC - 1),
                    )
                    if k + LEAD < len(iters):
                        emit_st(k + LEAD)

                # free the psum banks fast: scalar Ln evacuates the sums
                # (rec = Exp(-Ln(sums)) later, sharing the natural_log_exp
                # ACT table with the softmax Exps), vector copies evacuate
                # p1/p2. The normalization itself runs lazily at head end.
                lnt_r = pSm.tile([128, 512], F32, tag="sm")
                nc.scalar.activation(
                    lnt_r, sums_r, func=mybir.ActivationFunctionType.Ln
                )
                lnt_i = pSm.tile([128, 512], F32, tag="sm")
                nc.scalar.activation(
                    lnt_i, sums_i, func=mybir.ActivationFunctionType.Ln
                )
                p1c = pSm.tile([128, 512], F32, tag="sm")
                nc.vector.tensor_copy(p1c, p1)
                p2c = pSm.tile([128, 512], F32, tag="sm")
                nc.vector.tensor_copy(p2c, p2)
                norm.append((nsl, lnt_r, lnt_i, p1c, p2c))

            # deferred normalization (SBUF-only, off every critical path)
            for nsl, lnt_r, lnt_i, p1c, p2c in norm:
                nc.scalar.activation(
                    lnt_r, lnt_r, func=mybir.ActivationFunctionType.Exp, scale=-1.0
                )
                nc.scalar.activation(
                    lnt_i, lnt_i, func=mybir.ActivationFunctionType.Exp, scale=-1.0
                )
                t2 = pSm.tile([128, 512], F32, tag="sm")
                nc.vector.tensor_mul(ot[:, nsl], p1c, lnt_r)
                nc.vector.tensor_mul(t2, p2c, lnt_i)
                nc.vector.tensor_add(ot[:, nsl], ot[:, nsl], t2)
            osb.append(ot)

        # ---- output projection ----
        wo = []
        for h in range(H):
            t = pA.tile([128, 1024], MDT, tag="bigA")
            nc.sync.dma_start(out=t, in_=d_wo[h])
            wo.append(t)
        for t_ in range(TC):
            tsl = slice(t_ * 128, (t_ + 1) * 128)
            for nh in range(2):
                nsl = slice(nh * 512, (nh + 1) * 512)
                ps = ps_proj.tile([128, 512], F32, tag="ps_sums")
                for h in range(H):
                    nc.tensor.matmul(
                        ps,
                        lhsT=osb[h][:, tsl],
                        rhs=wo[h][:, nsl],
                        start=(h == 0),
                        stop=(h == H - 1),
                    )
                oev = pB.tile([128, 512], F32, tag="bigB")
                nc.scalar.copy(oev, ps)
                nc.sync.dma_start(out=d_out[tsl, nsl], in_=oev)

    _split_waits(nc)
    return nc


_NC_CACHE = {}


def kernel(
    queries,
    keys,
    values,
    wq_r,
    wq_i,
    wk_r,
    wk_i,
    wv_r,
    wv_i,
    wo_r,
    wo_i,
    _trace=False,
):
    global LAST_EXEC_NS
    _install_axon_profile_shim()
    _install_tile_drain_patch()
    from concourse.bass_utils import run_bass_kernel_spmd

    scale = 1.0 / np.sqrt(DH)
    WQ = _head_tiles(_build_wqk(np.asarray(wq_r), np.asarray(wq_i), scale))
    WK = _head_tiles(_build_wqk(np.asarray(wk_r), np.asarray(wk_i), 1.0))
    WV = _kchunk_tiles(_build_wqk(np.asarray(wv_r), np.asarray(wv_i), 1.0))
    WO = _kchunk_tiles(_build_wo(np.asarray(wo_r), np.asarray(wo_i)))
    CST = np.zeros((128, 320), np.float32)
    CST[:, 0:128] = 1.0

    queries = np.asarray(queries)
    keys = np.asarray(keys)
    values = np.asarray(values)

    in_maps = []
    for b in range(NCORES):
        in_maps.append(
            {
                "xtq": _xt(queries[b]).reshape(KC, 128, S),
                "xtk": _xt(keys[b]).reshape(KC, 128, S),
                "xtv": _xt(values[b]).reshape(KC, 128, S),
                "wq": WQ,
                "wk": WK,
                "wv": WV,
                "wo": WO,
                "cst": CST,
            }
        )

    if "nc" not in _NC_CACHE:
        _NC_CACHE["nc"] = _build_nc()
    nc = _NC_CACHE["nc"]

    res = run_bass_kernel_spmd(nc, in_maps, list(range(NCORES)), trace=_trace)
    LAST_EXEC_NS = res.exec_time_ns

    out = np.empty((B, S, D, 2), np.float32)
    for b in range(NCORES):
        out[b] = res.results[b]["out"].reshape(S, D, 2)
    return out



# revision 17
# speedup vs baseline: 1.1898x; 1.0211x over previous
"""Complex multi-head attention on 8 Trainium2 cores (Bass/Tile).

Sharding: pure data-parallel over batch (B=8 -> 1 batch per core),
weights replicated. No collectives.

Per-core dataflow (batch b), all matmuls float32r (full rate at N=512):
  - Host supplies feature-major activations XT = [xr.T; xi.T] [1024, S]
    and repacked/sign-folded weights so every complex linear is one
    stacked-K real matmul chain.
  - V-projection (all heads) -> V1 token-major [t, (h, vr|vi)].
  - Per head h: Q/K projections -> feature-major stacks [(c,dh)=128, S];
    scores computed TRANSPOSED (S.T = K-stationary) so softmax'd scores
    feed the AV matmul directly (no transposes anywhere);
    softmax without max-subtraction (|s| <= ~16, exp safe in fp32);
    row sums via ones-matmuls packed into one PSUM bank (tile_position);
    normalization fused into the P1/P2 PSUM evacuation via
    broadcast-DMA'd reciprocals.
  - Output projection accumulates heads as K-chunks -> [t, (o, c)] which
    is exactly the [S, D, 2] DRAM layout.
"""

import sys
import types
import numpy as np

B, S, D, H = 8, 1024, 512, 8
DH = D // H
KC = 8  # k-chunks of 128 over (c,d) = 1024
TC = 8  # token chunks of 128
NCORES = 8

LAST_EXEC_NS = None


# ---------------------------------------------------------------- shims
def _install_axon_profile_shim():
    if "antenv.axon_hooks" in sys.modules:
        return
    try:
        import antenv  # noqa: F401

        mod = types.ModuleType("antenv.axon_hooks")
        state = {"hook": None}
        mod.set_axon_ntff_profile_hook = lambda h: state.__setitem__("hook", h)
        mod.get_axon_ntff_profile_hook = lambda: state["hook"]
        sys.modules["antenv.axon_hooks"] = mod
        from trn_agent_boot.trn_boot import _ntff_profile_via_ctypes

        hook = _ntff_profile_via_ctypes("/opt/axon/libaxon_pjrt.so")
        if hook is not None:
            mod.set_axon_ntff_profile_hook(hook)
    except Exception:
        pass


def _install_tile_drain_patch():
    """This walrus build allows ONE sync wait per instruction; split the
    TileContext exit drain's waits across preceding sync NOPs."""
    import concourse.mybir as mybir
    import concourse.tile as tile
    from concourse.vector_clock import ScopedClock

    if getattr(tile.TileContext, "_drain_patched", False):
        return

    def _patched(self, tick_clock, wait_clock):
        probe = mybir.InstNoOp(name="I-drain-probe")
        probe.engine = mybir.EngineType.SP
        wait_clock.add_sem_waits(probe, ScopedClock({None: tick_clock.global_clock}))
        waits = list(probe.sync_info.on_wait or []) if probe.sync_info else []
        for w in waits:
            nop = self.nc.sync.nop()
            nop.ins.sync_info = mybir.SyncInfo(on_wait=[w], on_update=[])
        self.nc.sync.drain()
        self.nc.all_engine_barrier()
        assert self.sems is not None
        popped = self.nc._tile_sem_poison_stack.pop()
        assert popped is self._sem_poison
        self.nc.clear_and_free_semaphores(list(self.sems.allocated().values()))
        self.nc.all_engine_barrier()

    tile.TileContext._drain_and_barrier = _patched
    tile.TileContext._drain_patched = True


def _split_waits(nc, max_waits=1):
    """Hoist extra sync waits onto preceding same-engine NOPs (walrus here
    rejects >1 sync wait per instruction)."""
    import concourse.mybir as mybir

    def process(blk):
        lst = blk.instructions
        i = 0
        while i < len(lst):
            inst = lst[i]
            if hasattr(inst, "blocks"):
                for b in inst.blocks or []:
                    process(b)
            si = inst.sync_info
            if si is not None and si.on_wait and len(si.on_wait) > max_waits:
                waits = list(si.on_wait)
                keep, extra = waits[-max_waits:], waits[:-max_waits]
                inst.sync_info = mybir.SyncInfo(
                    on_wait=keep, on_update=list(si.on_update or [])
                )
                for j, w in enumerate(extra):
                    nop = mybir.InstNoOp(name=f"{inst.name}-ws{j}")
                    nop.engine = inst.engine
                    nop.sync_info = mybir.SyncInfo(on_wait=[w], on_update=[])
                    lst.insert(i, nop)
                    i += 1
            i += 1

    for f in nc.m.functions:
        for blk in f.blocks:
            process(blk)


# ------------------------------------------------------------ host prep
def _build_wqk(wr, wi, scale):
    """[1024 k=(c,d), 1024 m=(h, c', dh)] for Q/K projections."""
    W = np.empty((2 * D, 2 * D), np.float32)
    for h in range(H):
        o = slice(h * DH, (h + 1) * DH)
        c0 = h * 2 * DH
        W[0:D, c0 : c0 + DH] = wr[o].T * scale
        W[D:, c0 : c0 + DH] = -wi[o].T * scale
        W[0:D, c0 + DH : c0 + 2 * DH] = wi[o].T * scale
        W[D:, c0 + DH : c0 + 2 * DH] = wr[o].T * scale
    return W


def _head_tiles(W):
    """[1024,1024] -> [H, 128, 1024]: per-head column block, k-chunk cols."""
    out = np.empty((H, 128, 1024), np.float32)
    for h in range(H):
        blk = W[:, h * 128 : (h + 1) * 128]  # [1024, 128]
        for kk in range(KC):
            out[h, :, kk * 128 : (kk + 1) * 128] = blk[kk * 128 : (kk + 1) * 128]
    return out


def _kchunk_tiles(W):
    """[1024,1024] -> [KC, 128, 1024]: row chunks."""
    return np.ascontiguousarray(W.reshape(KC, 128, 1024))


def _build_wo(wo_r, wo_i):
    """rows (h, c', dh), cols (o, c) interleaved to match [S, D, 2]."""
    W = np.empty((2 * D, 2 * D), np.float32)
    for h in range(H):
        d = slice(h * DH, (h + 1) * DH)
        r0 = h * 2 * DH
        W[r0 : r0 + DH, 0::2] = wo_r[:, d].T
        W[r0 : r0 + DH, 1::2] = wo_i[:, d].T
        W[r0 + DH : r0 + 2 * DH, 0::2] = -wo_i[:, d].T
        W[r0 + DH : r0 + 2 * DH, 1::2] = wo_r[:, d].T
    return W


def _xt(x):  # [S, D, 2] -> [2D, S] feature-major
    out = np.empty((2 * D, S), np.float32)
    out[0:D] = x[:, :, 0].T
    out[D:] = x[:, :, 1].T
    return out


# ------------------------------------------------------------ bass build
def _build_nc():
    import concourse.bass as bass
    import concourse.bass as bass_mod
    import concourse.mybir as mybir
    import concourse.tile as tile
    from contextlib import ExitStack

    MDT = mybir.dt.float32r
    F32 = mybir.dt.float32

    nc = bass.Bass()
    d_xtq = nc.dram_tensor("xtq", [KC, 128, S], MDT, kind="ExternalInput")
    d_xtk = nc.dram_tensor("xtk", [KC, 128, S], MDT, kind="ExternalInput")
    d_xtv = nc.dram_tensor("xtv", [KC, 128, S], MDT, kind="ExternalInput")
    d_wq = nc.dram_tensor("wq", [H, 128, 1024], MDT, kind="ExternalInput")
    d_wk = nc.dram_tensor("wk", [H, 128, 1024], MDT, kind="ExternalInput")
    d_wv = nc.dram_tensor("wv", [KC, 128, 1024], MDT, kind="ExternalInput")
    d_wo = nc.dram_tensor("wo", [H, 128, 1024], MDT, kind="ExternalInput")
    d_cst = nc.dram_tensor("cst", [128, 320], MDT, kind="ExternalInput")
    d_out = nc.dram_tensor("out", [S, 1024], F32, kind="ExternalOutput")

    with tile.TileContext(nc) as tc, ExitStack() as ctx:
        ctx.enter_context(
            nc.allow_low_precision(reason="float32r tiles are bit-identical fp32")
        )
        pA = ctx.enter_context(tc.tile_pool(name="bigA", bufs=8))  # xtv -> xtq -> wo
        pB = ctx.enter_context(tc.tile_pool(name="bigB", bufs=8))  # wv -> xtk -> oev
        pV1 = ctx.enter_context(tc.tile_pool(name="v1", bufs=8))
        pOsb = ctx.enter_context(tc.tile_pool(name="osb", bufs=8))
        pWqk = ctx.enter_context(tc.tile_pool(name="wqk", bufs=3))
        pStk = ctx.enter_context(tc.tile_pool(name="stk", bufs=8))
        pE = ctx.enter_context(tc.tile_pool(name="e", bufs=4))
        pV2 = ctx.enter_context(tc.tile_pool(name="v2", bufs=10))
        pSm = ctx.enter_context(tc.tile_pool(name="sm", bufs=10))
        pC = ctx.enter_context(tc.tile_pool(name="const", bufs=1))

        # PSUM: 8 banks. st pipeline (3, lead-2 software pipeline), attention
        # accumulators p1/p2 (3: one slack bank absorbs the lazy norm),
        # sums + every projection psum (2).
        ps_st = ctx.enter_context(tc.tile_pool(name="ps_st", bufs=3, space="PSUM"))
        ps_p12 = ctx.enter_context(tc.tile_pool(name="ps_p12", bufs=3, space="PSUM"))
        ps_sums = ctx.enter_context(tc.tile_pool(name="ps_sums", bufs=2, space="PSUM"))
        
        cst = pC.tile([128, 320], MDT, tag="cst")
        nc.sync.dma_start(out=cst, in_=d_cst[:, :])
        ones128 = cst[:, 0:128]

        # ---- phase V: V projection (all heads) ----
        xtv = []
        for kk in range(KC):
            t = pA.tile([128, S], MDT, tag="bigA")
            nc.sync.dma_start(out=t, in_=d_xtv[kk])
            xtv.append(t)
        wv = []
        for kk in range(KC):
            t = pB.tile([128, 1024], MDT, tag="bigB")
            nc.sync.dma_start(out=t, in_=d_wv[kk])
            wv.append(t)
        v1 = []
        for t_ in range(TC):
            vt = pV1.tile([128, 1024], MDT, tag="v1")
            for nh in range(2):
                ps = ps_p12.tile([128, 512], F32, tag="ps_p12")
                for kk in range(KC):
                    nc.tensor.matmul(
                        ps,
                        lhsT=xtv[kk][:, t_ * 128 : (t_ + 1) * 128],
                        rhs=wv[kk][:, nh * 512 : (nh + 1) * 512],
                        start=(kk == 0),
                        stop=(kk == KC - 1),
                    )
                nc.vector.tensor_copy(vt[:, nh * 512 : (nh + 1) * 512], ps)
            v1.append(vt)

        # ---- load XT_q / XT_k (reuse pA / pB slots) ----
        xtq, xtk = [], []
        for kk in range(KC):
            t = pA.tile([128, S], MDT, tag="bigA")
            nc.sync.dma_start(out=t, in_=d_xtq[kk])
            xtq.append(t)
        for kk in range(KC):
            t = pB.tile([128, S], MDT, tag="bigB")
            nc.sync.dma_start(out=t, in_=d_xtk[kk])
            xtk.append(t)

        # ---- attention per head ----
        # Head h+1's Q/K projections are interleaved into head h's attention
        # (Q-proj between the nh groups, K-proj after nh1) so their psum
        # evacuations complete long before head h+1's first score matmul.
        osb = []
        wq_t, wk_t = {}, {}

        def dma_w(h):
            if h >= H:
                return
            t = pWqk.tile([128, 1024], MDT, tag="wqk")
            nc.sync.dma_start(out=t, in_=d_wq[h])
            wq_t[h] = t
            t = pWqk.tile([128, 1024], MDT, tag="wqk")
            nc.sync.dma_start(out=t, in_=d_wk[h])
            wk_t[h] = t

        def emit_qproj(h):
            # Q projection -> qstack [(c,dh)=128, S]
            qstack = pStk.tile([128, S], MDT, tag="stk")
            for nh in range(2):
                ps = ps_p12.tile([128, 512], F32, tag="ps_p12")
                for kk in range(KC):
                    nc.tensor.matmul(
                        ps,
                        lhsT=wq_t[h][:, kk * 128 : (kk + 1) * 128],
                        rhs=xtq[kk][:, nh * 512 : (nh + 1) * 512],
                        start=(kk == 0),
                        stop=(kk == KC - 1),
                    )
                nc.vector.tensor_copy(qstack[:, nh * 512 : (nh + 1) * 512], ps)
            # qswap = [qi.T; qr.T] via partition-crossing SBUF->SBUF DMA
            qswap = pStk.tile([128, S], MDT, tag="stk")
            nc.sync.dma_start(out=qswap[0:64, :], in_=qstack[64:128, :])
            nc.sync.dma_start(out=qswap[64:128, :], in_=qstack[0:64, :])
            return qstack, qswap

        def emit_kproj(h):
            # K projection -> kstack [kr.T; ki.T], kneg [kr.T; -ki.T]
            kstack = pStk.tile([128, S], MDT, tag="stk")
            kneg = pStk.tile([128, S], MDT, tag="stk")
            for nh in range(2):
                sl = slice(nh * 512, (nh + 1) * 512)
                ps = ps_p12.tile([128, 512], F32, tag="ps_p12")
                for kk in range(KC):
                    nc.tensor.matmul(
                        ps,
                        lhsT=wk_t[h][:, kk * 128 : (kk + 1) * 128],
                        rhs=xtk[kk][:, nh * 512 : (nh + 1) * 512],
                        start=(kk == 0),
                        stop=(kk == KC - 1),
                    )
                nc.vector.tensor_copy(kstack[:, sl], ps)
                nc.vector.tensor_copy(kneg[0:64, sl], ps[0:64, :])
                nc.vector.tensor_scalar_mul(kneg[64:128, sl], ps[64:128, :], -1.0)
            return kstack, kneg

        dma_w(0)
        qstack_n, qswap_n = emit_qproj(0)
        kstack_n, kneg_n = emit_kproj(0)
        dma_w(1)

        for h in range(H):
            qstack, qswap = qstack_n, qswap_n
            kstack, kneg = kstack_n, kneg_n

            # V2_h tiles: [-vi | vr] per tk-chunk
            v2h = []
            for tk in range(TC):
                vt = pV2.tile([128, 128], MDT, tag="v2")
                base = h * 128
                nc.vector.tensor_scalar_mul(
                    vt[:, 0:64], v1[tk][:, base + 64 : base + 128], -1.0
                )
                nc.vector.tensor_copy(vt[:, 64:128], v1[tk][:, base : base + 64])
                v2h.append(vt)

            ot = pOsb.tile([128, S], MDT, tag="osb")
            norm = []  # deferred per-nh normalization state
            for nh in range(2):
                nsl = slice(nh * 512, (nh + 1) * 512)
                sums_r = ps_sums.tile([128, 512], F32, tag="ps_sums")
                sums_i = ps_sums.tile([128, 512], F32, tag="ps_sums")
                p1 = ps_p12.tile([128, 512], F32, tag="ps_p12")
                p2 = ps_p12.tile([128, 512], F32, tag="ps_p12")

                # software-pipelined: score matmuls run LEAD iterations ahead
                # so the in-order tensor queue never blocks on Exp latency.
                iters = [(tk, comp) for tk in range(TC) for comp in range(2)]
                st_tiles = [None] * len(iters)

                def emit_st(k):
                    tk, comp = iters[k]
                    ksl = slice(tk * 128, (tk + 1) * 128)
                    st = ps_st.tile([128, 512], F32, tag="ps_st")
                    nc.tensor.matmul(
                        st,
                        lhsT=(kneg if comp == 0 else kstack)[:, ksl],
                        rhs=(qstack if comp == 0 else qswap)[:, nsl],
                        start=True,
                        stop=True,
                    )
                    st_tiles[k] = st

                LEAD = 2
                for k in range(LEAD):
                    emit_st(k)
                for k in range(len(iters)):
                    tk, comp = iters[k]
                    e = pE.tile([128, 512], MDT, tag="e")
                    nc.scalar.activation(
                        e, st_tiles[k], func=mybir.ActivationFunctionType.Exp
                    )
                    st_tiles[k] = None
                    sdst = sums_r if comp == 0 else sums_i
                    pdst = p1 if comp == 0 else p2
                    vt = v1[tk][:, h * 128 : (h + 1) * 128] if comp == 0 else v2h[tk]
                    nc.tensor.matmul(
                        sdst,
                        lhsT=ones128,
                        rhs=e,
                        start=(tk == 0),
                        stop=(tk == TC - 1),
                    )
                    nc.tensor.matmul(
                        pdst,
                        lhsT=vt,
                        rhs=e,
                        start=(tk == 0),
                        stop=(tk == TC - 1),
                    )
                    if k + LEAD < len(iters):
                        emit_st(k + LEAD)

                # free the psum banks fast: scalar Ln evacuates the sums
                # (rec = Exp(-Ln(sums)) later, sharing the natural_log_exp
                # ACT table with the softmax Exps), vector copies evacuate
                # p1/p2. The normalization itself runs lazily at head end.
                lnt_r = pSm.tile([128, 512], F32, tag="sm")
                nc.scalar.activation(
                    lnt_r, sums_r, func=mybir.ActivationFunctionType.Ln
                )
                lnt_i = pSm.tile([128, 512], F32, tag="sm")
                nc.scalar.activation(
                    lnt_i, sums_i, func=mybir.ActivationFunctionType.Ln
                )
                p1c = pSm.tile([128, 512], F32, tag="sm")
                nc.vector.tensor_copy(p1c, p1)
                p2c = pSm.tile([128, 512], F32, tag="sm")
                nc.vector.tensor_copy(p2c, p2)
                norm.append((nsl, lnt_r, lnt_i, p1c, p2c))

                # interleave next head's projections into this head's stream
                if nh == 0 and h + 1 < H:
                    qstack_n, qswap_n = emit_qproj(h + 1)

            if h + 1 < H:
                kstack_n, kneg_n = emit_kproj(h + 1)
                dma_w(h + 2)

            # deferred normalization (SBUF-only, off every critical path)
            for nsl, lnt_r, lnt_i, p1c, p2c in norm:
                nc.scalar.activation(
                    lnt_r, lnt_r, func=mybir.ActivationFunctionType.Exp, scale=-1.0
                )
                nc.scalar.activation(
                    lnt_i, lnt_i, func=mybir.ActivationFunctionType.Exp, scale=-1.0
                )
                t2 = pSm.tile([128, 512], F32, tag="sm")
                nc.vector.tensor_mul(ot[:, nsl], p1c, lnt_r)
                nc.vector.tensor_mul(t2, p2c, lnt_i)
                nc.vector.tensor_add(ot[:, nsl], ot[:, nsl], t2)
            osb.append(ot)

        # ---- output projection ----
        wo = []
        for h in range(H):
            t = pA.tile([128, 1024], MDT, tag="bigA")
            nc.sync.dma_start(out=t, in_=d_wo[h])
            wo.append(t)
        for t_ in range(TC):
            tsl = slice(t_ * 128, (t_ + 1) * 128)
            for nh in range(2):
                nsl = slice(nh * 512, (nh + 1) * 512)
                ps = ps_p12.tile([128, 512], F32, tag="ps_p12")
                for h in range(H):
                    nc.tensor.matmul(
                        ps,
                        lhsT=osb[h][:, tsl],
                        rhs=wo[h][:, nsl],
                        start=(h == 0),
                        stop=(h == H - 1),
                    )
                oev = pB.tile([128, 512], F32, tag="bigB")
                nc.scalar.copy(oev, ps)
                nc.sync.dma_start(out=d_out[tsl, nsl], in_=oev)

    _split_waits(nc)
    return nc


_NC_CACHE = {}


def kernel(
    queries,
    keys,
    values,
    wq_r,
    wq_i,
    wk_r,
    wk_i,
    wv_r,
    wv_i,
    wo_r,
    wo_i,
    _trace=False,
):
    global LAST_EXEC_NS
    _install_axon_profile_shim()
    _install_tile_drain_patch()
    from concourse.bass_utils import run_bass_kernel_spmd

    scale = 1.0 / np.sqrt(DH)
    WQ = _head_tiles(_build_wqk(np.asarray(wq_r), np.asarray(wq_i), scale))
    WK = _head_tiles(_build_wqk(np.asarray(wk_r), np.asarray(wk_i), 1.0))
    WV = _kchunk_tiles(_build_wqk(np.asarray(wv_r), np.asarray(wv_i), 1.0))
    WO = _kchunk_tiles(_build_wo(np.asarray(wo_r), np.asarray(wo_i)))
    CST = np.zeros((128, 320), np.float32)
    CST[:, 0:128] = 1.0

    queries = np.asarray(queries)
    keys = np.asarray(keys)
    values = np.asarray(values)

    in_maps = []
    for b in range(NCORES):
        in_maps.append(
            {
                "xtq": _xt(queries[b]).reshape(KC, 128, S),
                "xtk": _xt(keys[b]).reshape(KC, 128, S),
                "xtv": _xt(values[b]).reshape(KC, 128, S),
                "wq": WQ,
                "wk": WK,
                "wv": WV,
                "wo": WO,
                "cst": CST,
            }
        )

    if "nc" not in _NC_CACHE:
        _NC_CACHE["nc"] = _build_nc()
    nc = _NC_CACHE["nc"]

    res = run_bass_kernel_spmd(nc, in_maps, list(range(NCORES)), trace=_trace)
    LAST_EXEC_NS = res.exec_time_ns

    out = np.empty((B, S, D, 2), np.float32)
    for b in range(NCORES):
        out[b] = res.results[b]["out"].reshape(S, D, 2)
    return out

